# revision 4
# baseline (speedup 1.0000x reference)
"""Binarized Conv1d + BatchNorm1d (training mode) on 8 TRN2 NeuronCores.

Reference computation:
    bx  = sign(x)          [B=16, Cin=128, L=8192]
    bw  = sign(weight)     [Cout=128, Cin=128, K=5]
    out = conv1d(bx, bw, stride=1, pad=2) + bias
    out = (out - mean(out, (B,L))) * rsqrt(var(out, (B,L)) + 1e-5)

Sharding: data-parallel over batch, 2 batches per core; weights replicated.

Key tricks vs the straightforward version:
  - step encoding: s = step(x) in {0,1} (pad cols = 0.5).  Then the true
    conv is 2*conv(s,bw) - C[co] with C constant per channel, and C (like
    the conv bias) cancels inside training-mode BN.  So the kernel only
    computes M = conv(s,bw) and normalizes with
        out = (M - mean_M) * rsqrt(var_M + EPS/4).
    step() is a single is_gt op (exact: this input has no x==0), so the
    f32->fp8 binarize runs on DVE/ACT/GpSimd concurrently.
  - fp8 DoubleRow matmuls: two conv taps per PE pass (2x bf16 rate).
    s and sign(w) are exact in fp8e4; PSUM accumulates f32, so conv
    is exact.
  - weights are sign()ed/transposed to [ci,k,co] fp8 on the host.
  - stats all-reduce via a [2,128]-transposed AllGather: the gathered
    [16,128] reduces with one tiny matmul (no 1024-descriptor DMAs).
"""

import os
import sys

import numpy as np

try:
    import concourse  # noqa: F401
except ImportError:
    for _p in ("/opt/trn_rl_repo", "/root/.axon_site/_ro/trn_rl_repo"):
        if os.path.isdir(_p):
            sys.path.insert(0, _p)
            break

B = 16
B_LOC = 2
CI = 128
CO = 128
L = 8192
K = 5
PAD = 2
EPS = 1e-5
N_CORES = 8
FREE = 512          # PSUM tile free dim (one bank of f32)
NT = L // FREE      # 16 conv tiles per batch row
WARMUP_CC = True    # fire a dummy collective early to absorb CC setup

_CACHE = {}


def _win2(bxp, base, n=FREE):
    """Overlapping DoubleRow rhs window: [128, 2, n] where slot i reads
    bxp[:, base+i : base+i+n] (taps k and k+1 share the buffer)."""
    u = bxp[:, base : base + n].unsqueeze(1)
    u.ap[1] = [1, 2]
    return u


def _build_nc():
    import concourse.bacc as bacc
    import concourse.tile as tile
    from concourse import mybir

    f32 = mybir.dt.float32
    f8 = mybir.dt.float8e4
    Sigmoid = mybir.ActivationFunctionType.Sigmoid
    Sqrt = mybir.ActivationFunctionType.Sqrt
    Copy = mybir.ActivationFunctionType.Copy
    Ident = mybir.ActivationFunctionType.Identity
    ALU = mybir.AluOpType
    DR = mybir.MatmulPerfMode.DoubleRow

    nc = bacc.Bacc("TRN2", target_bir_lowering=False, debug=False, num_devices=N_CORES)

    x = nc.declare_dram_parameter("x", [B_LOC, CI, L], f32, isOutput=False)
    wT = nc.declare_dram_parameter("wT", [CI, K, CO], f8, isOutput=False)
    idm = nc.declare_dram_parameter("ident", [128, 128], f32, isOutput=False)
    sel = nc.declare_dram_parameter("sel", [16, 2], f32, isOutput=False)
    out = nc.declare_dram_parameter("out", [B_LOC, CO, L], f32, isOutput=True)

    with tile.TileContext(nc) as tc:
        with (
            tc.tile_pool(name="singles", bufs=1) as singles,
            tc.tile_pool(name="xin", bufs=1) as xin,
            tc.tile_pool(name="bxp", bufs=2) as bxp_pool,
            tc.tile_pool(name="psum", bufs=8, space="PSUM") as psum,
            tc.tile_pool(name="dram", bufs=2, space="DRAM") as dram,
        ):
            # ---- warm-up collective: absorb cross-core rendezvous/setup
            # behind the conv phase ----
            if WARMUP_CC:
                warm_in = dram.tile([1, 8], f32)
                warm_out = dram.tile([N_CORES, 8], f32)
                nc.sync.dma_start(out=warm_in, in_=x[0, 0:1, 0:8])
                nc.gpsimd.collective_compute(
                    "AllGather",
                    mybir.AluOpType.bypass,
                    replica_groups=[list(range(N_CORES))],
                    ins=[warm_in[:].opt()],
                    outs=[warm_out[:].opt()],
                )

            # ---- constants + weights + x streamed in ----
            wTt = singles.tile([CI, K, CO], f8)
            nc.sync.dma_start(out=wTt, in_=wT[:, :, :])
            xts = [
                xin.tile([CI, L], f32, tag=f"xt{b}", name=f"xt{b}")
                for b in range(B_LOC)
            ]
            CHUNKS = {0: [512, 1536, 2048, 2048, 2048], 1: [2048, 2048, 2048, 2048]}
            for b in range(B_LOC):
                off = 0
                for ch in CHUNKS[b]:
                    nc.sync.dma_start(
                        out=xts[b][:, off : off + ch], in_=x[b, :, off : off + ch]
                    )
                    off += ch
            ident = singles.tile([128, 128], f32)
            nc.sync.dma_start(out=ident, in_=idm[:, :])
            sel_sb = singles.tile([16, 2], f32)
            nc.sync.dma_start(out=sel_sb, in_=sel[:, :])

            # ---- conv: binarize (step encoding) + fp8 DoubleRow matmuls ----
            conv_sb = singles.tile([CO, B_LOC, L], f32)
            stats = singles.tile([CO, B_LOC * NT, 6], f32)

            copy_idx = 0
            for b in range(B_LOC):
                bxp = bxp_pool.tile([CI, L + 2 * PAD], f8)
                nc.vector.memset(bxp[:, 0:PAD], 0.5)
                nc.vector.memset(bxp[:, L + PAD : L + 2 * PAD], 0.5)
                xt = xts[b]
                done_t = 0
                off = 0
                for ch in CHUNKS[b]:
                    # split the chunk across DVE / ACT / GpSimd
                    if ch <= 512:
                        splits = [(0, ch, "v")]
                    else:
                        d = ch - 1024
                        splits = [(0, 1024, "v"), (1024, d // 2, "a"),
                                  (1024 + d // 2, d - d // 2, "g")]
                    for s0, n, eng in splits:
                        if n <= 0:
                            continue
                        dst = bxp[:, PAD + off + s0 : PAD + off + s0 + n]
                        src = xt[:, off + s0 : off + s0 + n]
                        if eng == "v":
                            nc.vector.tensor_scalar(
                                out=dst, in0=src, scalar1=0.0, scalar2=None,
                                op0=ALU.is_gt,
                            )
                        elif eng == "a":
                            nc.scalar.activation(
                                out=dst, in_=src, func=Sigmoid, scale=1e30
                            )
                        else:
                            nc.gpsimd.tensor_scalar(
                                out=dst, in0=src, scalar1=0.0, scalar2=None,
                                op0=ALU.is_gt,
                            )
                    off += ch
                    # conv tiles fully covered by binarized cols [0, off)
                    # tile t needs bxp up to index t*512+515; filled thru
                    # 2+off-1 (plus right pad once off==L)
                    lim = off + PAD - 1 + (PAD if off == L else 0)
                    while done_t < NT and done_t * FREE + 515 <= lim:
                        t = done_t
                        pt = psum.tile([CO, FREE], f32, tag="pt")
                        base = t * FREE
                        nc.tensor.matmul(
                            pt, lhsT=wTt[:, 0:2, :], rhs=_win2(bxp, base),
                            start=True, stop=False, perf_mode=DR,
                        )
                        nc.tensor.matmul(
                            pt, lhsT=wTt[:, 2:4, :], rhs=_win2(bxp, base + 2),
                            start=False, stop=False, perf_mode=DR,
                        )
                        nc.tensor.matmul(
                            pt, lhsT=wTt[:, 4, :],
                            rhs=bxp[:, base + 4 : base + 4 + FREE],
                            start=False, stop=True,
                        )
                        nc.vector.bn_stats(out=stats[:, b * NT + t, :], in_=pt)
                        dst = conv_sb[:, b, t * FREE : (t + 1) * FREE]
                        if copy_idx % 2 == 0:
                            nc.scalar.activation(out=dst, in_=pt, func=Copy)
                        else:
                            nc.vector.tensor_copy(out=dst, in_=pt)
                        copy_idx += 1
                        done_t += 1

            # ---- local stats -> (mean, E[x^2]) transposed to [2,128] ----
            pk = singles.tile([CO, 2], f32)
            sq = singles.tile([CO, 1], f32)
            nc.vector.bn_aggr(out=pk, in_=stats)
            nc.vector.tensor_mul(sq, pk[:, 0:1], pk[:, 0:1])
            nc.vector.tensor_add(pk[:, 1:2], pk[:, 1:2], sq)
            ptp = psum.tile([2, CO], f32, tag="pt")
            nc.tensor.transpose(ptp, pk, ident)
            pkT = singles.tile([2, CO], f32)
            nc.vector.tensor_copy(out=pkT, in_=ptp)

            # ---- AllGather [2,128] -> [16,128]; matmul-reduce over cores ----
            cc_in = dram.tile([2, CO], f32)
            cc_out = dram.tile([2 * N_CORES, CO], f32)
            nc.sync.dma_start(out=cc_in, in_=pkT)
            nc.gpsimd.collective_compute(
                "AllGather",
                mybir.AluOpType.bypass,
                replica_groups=[list(range(N_CORES))],
                ins=[cc_in[:].opt()],
                outs=[cc_out[:].opt()],
            )
            gst16 = singles.tile([2 * N_CORES, CO], f32)
            nc.sync.dma_start(out=gst16, in_=cc_out[:, :])
            ps2 = psum.tile([2, CO], f32, tag="pt")
            nc.tensor.matmul(ps2, lhsT=sel_sb, rhs=gst16, start=True, stop=True)
            s2 = singles.tile([2, CO], f32)
            nc.vector.tensor_copy(out=s2, in_=ps2)
            pg = psum.tile([CO, 2], f32, tag="pt")
            nc.tensor.transpose(pg, s2, ident[0:2, 0:2])

            # a = rsqrt(var_M + EPS/4); shift = -mean_M * a
            gmean = singles.tile([CO, 1], f32)
            m2 = singles.tile([CO, 1], f32)
            gvar = singles.tile([CO, 1], f32)
            sd = singles.tile([CO, 1], f32)
            a_sc = singles.tile([CO, 1], f32)
            shift = singles.tile([CO, 1], f32)
            nc.vector.tensor_scalar_mul(gmean, pg[:, 0:1], 1.0 / N_CORES)
            nc.vector.tensor_mul(m2, gmean, gmean)
            nc.vector.tensor_scalar(
                out=gvar, in0=pg[:, 1:2], scalar1=1.0 / N_CORES,
                scalar2=m2[:, 0:1], op0=ALU.mult, op1=ALU.subtract,
            )
            eps_t = singles.tile([CO, 1], f32)
            nc.vector.memset(eps_t, EPS / 4.0)
            nc.scalar.activation(out=sd, in_=gvar, func=Sqrt, bias=eps_t[:, 0:1])
            nc.vector.reciprocal(a_sc, sd)
            nc.vector.tensor_scalar(
                out=shift, in0=gmean, scalar1=a_sc[:, 0:1], scalar2=-1.0,
                op0=ALU.mult, op1=ALU.mult,
            )

            # ---- normalize + store (DMA-bound; DVE/ACT/GpSimd produce) ----
            XCH = 1024
            ENG = ["v", "a", "v", "g", "v", "a", "v", "v",
                   "a", "v", "g", "v", "a", "v", "v", "a"]
            idx = 0
            for b in range(B_LOC):
                for c in range(L // XCH):
                    sl = conv_sb[:, b, c * XCH : (c + 1) * XCH]
                    eng = ENG[idx % len(ENG)]
                    if eng == "v":
                        nc.vector.tensor_scalar(
                            out=sl, in0=sl, scalar1=a_sc[:, 0:1],
                            scalar2=shift[:, 0:1], op0=ALU.mult, op1=ALU.add,
                        )
                    elif eng == "a":
                        nc.scalar.activation(
                            out=sl, in_=sl, func=Ident,
                            bias=shift[:, 0:1], scale=a_sc[:, 0:1],
                        )
                    else:
                        nc.gpsimd.tensor_scalar(
                            out=sl, in0=sl, scalar1=a_sc[:, 0:1],
                            scalar2=shift[:, 0:1], op0=ALU.mult, op1=ALU.add,
                        )
                    idx += 1
                    nc.sync.dma_start(
                        out=out[b, :, c * XCH : (c + 1) * XCH], in_=sl
                    )

    nc.compile()
    return nc


def _host_inputs(weight):
    from concourse import mybir

    f8np = mybir.dt.np(mybir.dt.float8e4)
    wT = np.sign(weight).transpose(1, 2, 0).astype(f8np)  # [ci, k, co]
    ident = np.eye(128, dtype=np.float32)
    sel = np.zeros((16, 2), dtype=np.float32)
    sel[0::2, 0] = 1.0  # even rows: per-core mean
    sel[1::2, 1] = 1.0  # odd rows: per-core E[x^2]
    return np.ascontiguousarray(wT), ident, sel


def _run(inputs, trace=False):
    from concourse import bass_utils

    x = np.ascontiguousarray(np.asarray(inputs["x"], dtype=np.float32))
    weight = np.ascontiguousarray(np.asarray(inputs["weight"], dtype=np.float32))

    if "nc" not in _CACHE:
        _CACHE["nc"] = _build_nc()
    nc = _CACHE["nc"]

    wT, ident, sel = _host_inputs(weight)
    in_maps = [
        {
            "x": x[i * B_LOC : (i + 1) * B_LOC],
            "wT": wT,
            "ident": ident,
            "sel": sel,
        }
        for i in range(N_CORES)
    ]
    res = bass_utils.run_bass_kernel_spmd(
        nc, in_maps, core_ids=list(range(N_CORES)), trace=trace
    )
    out = np.concatenate(
        [res.results[i]["out"] for i in range(N_CORES)], axis=0
    ).astype(np.float32)
    return out, res


def kernel(**inputs) -> np.ndarray:
    out, _ = _run(inputs, trace=False)
    return out


# revision 9
# speedup vs baseline: 1.4088x; 1.4088x over previous
"""Binarized Conv1d + BatchNorm1d (training mode) on 8 TRN2 NeuronCores.

Reference computation:
    bx  = sign(x)          [B=16, Cin=128, L=8192]
    bw  = sign(weight)     [Cout=128, Cin=128, K=5]
    out = conv1d(bx, bw, stride=1, pad=2) + bias
    out = (out - mean(out, (B,L))) * rsqrt(var(out, (B,L)) + 1e-5)

Sharding: data-parallel over batch, 2 batches per core; weights replicated.

Key tricks vs the straightforward version:
  - step encoding: s = step(x) in {0,1} (pad cols = 0.5).  Then the true
    conv is 2*conv(s,bw) - C[co] with C constant per channel, and C (like
    the conv bias) cancels inside training-mode BN.  So the kernel only
    computes M = conv(s,bw) and normalizes with
        out = (M - mean_M) * rsqrt(var_M + EPS/4).
    step() is a single is_gt op (exact: this input has no x==0), so the
    f32->bf16 binarize runs on DVE and ACT concurrently (fp8 output on
    DVE/GpSimd measured ~20x slow-path; fp8 matmul gave no PE win).
  - weights are sign()ed/transposed to [ci,k,co] bf16 on the host.
  - stats all-reduce via a [2,128]-transposed AllGather: the gathered
    [16,128] reduces with one tiny matmul (no 1024-descriptor DMAs).
"""

import os
import sys

import numpy as np

try:
    import concourse  # noqa: F401
except ImportError:
    for _p in ("/opt/trn_rl_repo", "/root/.axon_site/_ro/trn_rl_repo"):
        if os.path.isdir(_p):
            sys.path.insert(0, _p)
            break

B = 16
B_LOC = 2
CI = 128
CO = 128
L = 8192
K = 5
PAD = 2
EPS = 1e-5
N_CORES = 8
FREE = 512          # PSUM tile free dim (one bank of f32)
NT = L // FREE      # 16 conv tiles per batch row
WARMUP_CC = True    # fire a dummy collective early to absorb CC setup

_CACHE = {}


def _build_nc():
    import concourse.bacc as bacc
    import concourse.tile as tile
    from concourse import mybir

    f32 = mybir.dt.float32
    bf16 = mybir.dt.bfloat16
    Sigmoid = mybir.ActivationFunctionType.Sigmoid
    Sqrt = mybir.ActivationFunctionType.Sqrt
    Copy = mybir.ActivationFunctionType.Copy
    Ident = mybir.ActivationFunctionType.Identity
    ALU = mybir.AluOpType

    nc = bacc.Bacc("TRN2", target_bir_lowering=False, debug=False, num_devices=N_CORES)

    x = nc.declare_dram_parameter("x", [B_LOC, CI, L], f32, isOutput=False)
    wT = nc.declare_dram_parameter("wT", [CI, K, CO], bf16, isOutput=False)
    idm = nc.declare_dram_parameter("ident", [128, 128], f32, isOutput=False)
    sel = nc.declare_dram_parameter("sel", [16, 2], f32, isOutput=False)
    out = nc.declare_dram_parameter("out", [B_LOC, CO, L], f32, isOutput=True)

    with tile.TileContext(nc) as tc:
        with (
            tc.tile_pool(name="singles", bufs=1) as singles,
            tc.tile_pool(name="xin", bufs=1) as xin,
            tc.tile_pool(name="bxp", bufs=2) as bxp_pool,
            tc.tile_pool(name="psum", bufs=8, space="PSUM") as psum,
            tc.tile_pool(name="dram", bufs=2, space="DRAM") as dram,
        ):
            # ---- warm-up collective: absorb cross-core rendezvous/setup
            # behind the conv phase ----
            if WARMUP_CC:
                warm_in = dram.tile([1, 8], f32)
                warm_out = dram.tile([N_CORES, 8], f32)
                nc.sync.dma_start(out=warm_in, in_=x[0, 0:1, 0:8])
                nc.gpsimd.collective_compute(
                    "AllGather",
                    mybir.AluOpType.bypass,
                    replica_groups=[list(range(N_CORES))],
                    ins=[warm_in[:].opt()],
                    outs=[warm_out[:].opt()],
                )

            # ---- constants + weights + x streamed in ----
            wTt = singles.tile([CI, K, CO], bf16)
            nc.sync.dma_start(out=wTt, in_=wT[:, :, :])
            xts = [
                xin.tile([CI, L], f32, tag=f"xt{b}", name=f"xt{b}")
                for b in range(B_LOC)
            ]
            CHUNKS = {0: [512, 1536, 2048, 2048, 2048], 1: [2048, 2048, 2048, 2048]}
            for b in range(B_LOC):
                off = 0
                for ch in CHUNKS[b]:
                    nc.sync.dma_start(
                        out=xts[b][:, off : off + ch], in_=x[b, :, off : off + ch]
                    )
                    off += ch
            ident = singles.tile([128, 128], f32)
            nc.sync.dma_start(out=ident, in_=idm[:, :])
            sel_sb = singles.tile([16, 2], f32)
            nc.sync.dma_start(out=sel_sb, in_=sel[:, :])

            # ---- conv: binarize (step encoding) + fp8 DoubleRow matmuls ----
            conv_sb = singles.tile([CO, B_LOC, L], f32)
            stats = singles.tile([CO, B_LOC * NT, 6], f32)

            copy_idx = 0
            for b in range(B_LOC):
                bxp = bxp_pool.tile([CI, L + 2 * PAD], bf16)
                nc.vector.memset(bxp[:, 0:PAD], 0.5)
                nc.vector.memset(bxp[:, L + PAD : L + 2 * PAD], 0.5)
                xt = xts[b]
                done_t = 0
                off = 0
                for ch in CHUNKS[b]:
                    # split the chunk between DVE (is_gt) and ACT (sigmoid)
                    if ch <= 512:
                        splits = [(0, ch, "v")]
                    else:
                        d = (ch * 9 // 16) // 128 * 128  # DVE ~56%
                        splits = [(0, d, "v"), (d, ch - d, "a")]
                    for s0, n, eng in splits:
                        if n <= 0:
                            continue
                        dst = bxp[:, PAD + off + s0 : PAD + off + s0 + n]
                        src = xt[:, off + s0 : off + s0 + n]
                        if eng == "v":
                            nc.vector.tensor_scalar(
                                out=dst, in0=src, scalar1=0.0, scalar2=None,
                                op0=ALU.is_gt,
                            )
                        else:
                            nc.scalar.activation(
                                out=dst, in_=src, func=Sigmoid, scale=1e30
                            )
                    off += ch
                    # conv tiles fully covered by binarized cols [0, off)
                    # tile t needs bxp up to index t*512+515; filled thru
                    # 2+off-1 (plus right pad once off==L)
                    lim = off + PAD - 1 + (PAD if off == L else 0)
                    while done_t < NT and done_t * FREE + 515 <= lim:
                        t = done_t
                        pt = psum.tile([CO, FREE], f32, tag="pt")
                        base = t * FREE
                        for k in range(K):
                            nc.tensor.matmul(
                                pt, lhsT=wTt[:, k, :],
                                rhs=bxp[:, base + k : base + k + FREE],
                                start=(k == 0), stop=(k == K - 1),
                            )
                        nc.vector.bn_stats(out=stats[:, b * NT + t, :], in_=pt)
                        dst = conv_sb[:, b, t * FREE : (t + 1) * FREE]
                        if copy_idx % 2 == 0:
                            nc.scalar.activation(out=dst, in_=pt, func=Copy)
                        else:
                            nc.vector.tensor_copy(out=dst, in_=pt)
                        copy_idx += 1
                        done_t += 1

            # ---- local stats -> (mean, E[x^2]) transposed to [2,128] ----
            pk = singles.tile([CO, 2], f32)
            sq = singles.tile([CO, 1], f32)
            nc.vector.bn_aggr(out=pk, in_=stats)
            nc.vector.tensor_mul(sq, pk[:, 0:1], pk[:, 0:1])
            nc.vector.tensor_add(pk[:, 1:2], pk[:, 1:2], sq)
            ptp = psum.tile([2, CO], f32, tag="pt")
            nc.tensor.transpose(ptp, pk, ident)
            pkT = singles.tile([2, CO], f32)
            nc.vector.tensor_copy(out=pkT, in_=ptp)

            # ---- AllGather [2,128] -> [16,128]; matmul-reduce over cores ----
            cc_in = dram.tile([2, CO], f32)
            cc_out = dram.tile([2 * N_CORES, CO], f32)
            nc.sync.dma_start(out=cc_in, in_=pkT)
            nc.gpsimd.collective_compute(
                "AllGather",
                mybir.AluOpType.bypass,
                replica_groups=[list(range(N_CORES))],
                ins=[cc_in[:].opt()],
                outs=[cc_out[:].opt()],
            )
            gst16 = singles.tile([2 * N_CORES, CO], f32)
            nc.sync.dma_start(out=gst16, in_=cc_out[:, :])
            ps2 = psum.tile([2, CO], f32, tag="pt")
            nc.tensor.matmul(ps2, lhsT=sel_sb, rhs=gst16, start=True, stop=True)
            s2 = singles.tile([2, CO], f32)
            nc.vector.tensor_copy(out=s2, in_=ps2)
            pg = psum.tile([CO, 2], f32, tag="pt")
            nc.tensor.transpose(pg, s2, ident[0:2, 0:2])

            # a = rsqrt(var_M + EPS/4); shift = -mean_M * a
            gmean = singles.tile([CO, 1], f32)
            m2 = singles.tile([CO, 1], f32)
            gvar = singles.tile([CO, 1], f32)
            sd = singles.tile([CO, 1], f32)
            a_sc = singles.tile([CO, 1], f32)
            shift = singles.tile([CO, 1], f32)
            nc.vector.tensor_scalar_mul(gmean, pg[:, 0:1], 1.0 / N_CORES)
            nc.vector.tensor_mul(m2, gmean, gmean)
            nc.vector.tensor_scalar(
                out=gvar, in0=pg[:, 1:2], scalar1=1.0 / N_CORES,
                scalar2=m2[:, 0:1], op0=ALU.mult, op1=ALU.subtract,
            )
            eps_t = singles.tile([CO, 1], f32)
            nc.vector.memset(eps_t, EPS / 4.0)
            nc.scalar.activation(out=sd, in_=gvar, func=Sqrt, bias=eps_t[:, 0:1])
            nc.vector.reciprocal(a_sc, sd)
            nc.vector.tensor_scalar(
                out=shift, in0=gmean, scalar1=a_sc[:, 0:1], scalar2=-1.0,
                op0=ALU.mult, op1=ALU.mult,
            )

            # ---- normalize + store (DMA-bound; DVE/ACT/GpSimd produce) ----
            XCH = 1024
            ENG = ["v", "a", "v", "g", "v", "a", "v", "v",
                   "a", "v", "g", "v", "a", "v", "v", "a"]
            idx = 0
            for b in range(B_LOC):
                for c in range(L // XCH):
                    sl = conv_sb[:, b, c * XCH : (c + 1) * XCH]
                    eng = ENG[idx % len(ENG)]
                    if eng == "v":
                        nc.vector.tensor_scalar(
                            out=sl, in0=sl, scalar1=a_sc[:, 0:1],
                            scalar2=shift[:, 0:1], op0=ALU.mult, op1=ALU.add,
                        )
                    elif eng == "a":
                        nc.scalar.activation(
                            out=sl, in_=sl, func=Ident,
                            bias=shift[:, 0:1], scale=a_sc[:, 0:1],
                        )
                    else:
                        nc.gpsimd.tensor_scalar(
                            out=sl, in0=sl, scalar1=a_sc[:, 0:1],
                            scalar2=shift[:, 0:1], op0=ALU.mult, op1=ALU.add,
                        )
                    idx += 1
                    nc.sync.dma_start(
                        out=out[b, :, c * XCH : (c + 1) * XCH], in_=sl
                    )

    nc.compile()
    return nc


def _host_inputs(weight):
    from concourse import mybir

    bf16np = mybir.dt.np(mybir.dt.bfloat16)
    wT = np.sign(weight).transpose(1, 2, 0).astype(bf16np)  # [ci, k, co]
    ident = np.eye(128, dtype=np.float32)
    sel = np.zeros((16, 2), dtype=np.float32)
    sel[0::2, 0] = 1.0  # even rows: per-core mean
    sel[1::2, 1] = 1.0  # odd rows: per-core E[x^2]
    return np.ascontiguousarray(wT), ident, sel


def _run(inputs, trace=False):
    from concourse import bass_utils

    x = np.ascontiguousarray(np.asarray(inputs["x"], dtype=np.float32))
    weight = np.ascontiguousarray(np.asarray(inputs["weight"], dtype=np.float32))

    if "nc" not in _CACHE:
        _CACHE["nc"] = _build_nc()
    nc = _CACHE["nc"]

    wT, ident, sel = _host_inputs(weight)
    in_maps = [
        {
            "x": x[i * B_LOC : (i + 1) * B_LOC],
            "wT": wT,
            "ident": ident,
            "sel": sel,
        }
        for i in range(N_CORES)
    ]
    res = bass_utils.run_bass_kernel_spmd(
        nc, in_maps, core_ids=list(range(N_CORES)), trace=trace
    )
    out = np.concatenate(
        [res.results[i]["out"] for i in range(N_CORES)], axis=0
    ).astype(np.float32)
    return out, res


def kernel(**inputs) -> np.ndarray:
    out, _ = _run(inputs, trace=False)
    return out
